# revision 22
# baseline (speedup 1.0000x reference)
"""Trainium2 Bass kernel for nn_CortexBlock_59940563583556.

Math note (exact, not an approximation): the reference initializes the
fast-weight state U0 = V0 = 0 inside reference() itself, and every term
of the scan's update to U/V is proportional to ku = k_t^T @ U (zero when
U == 0).  By induction U_t == V_t == 0 for the whole scan, for ANY input
values.  Hence k_fast == 0, score_fast == 0, and (since mix_logit is
added to both logits, softmax is shift-invariant) the block reduces
exactly to:

    q = h @ Wq.T ; k = h @ Wk.T ; v = h @ Wv.T          (per-head split)
    g[b,t,h]  = sigmoid( sum_d q[b,t,h,d] * k[b,t,h,d] / sqrt(64) )
    out       = (g * v  per head) @ Wo.T

m_gate / alpha_scale / Wa / ba / mix_logit do not affect the output.

Sharding: data-parallel over the 8192 rows of the flattened [B*T, D]
activations (1024 rows/core); the four 1024x1024 weights are replicated.

Precision split: q/k only feed the sigmoid gate, so their GEMMs run in
fp8e4m3 with DoubleRow (2 fp8 MACs/cell/cycle -> half the PE time); the
value path (v, out) stays bf16 end-to-end.  Host-simulated worst-element
relative error 1.3e-2 vs the 2e-2 budget.  Inputs are pre-scaled on the
host (h x16, W x1024) to clear fp8's subnormal range; the product scale
is folded into the sigmoid's scale argument.  The DoubleRow moving
operand streams 2 elements/cycle only if each (ko=0, ko=1) pair sits in
adjacent bytes, so the fp8 weights are laid out ko-INNERMOST on the host
and fed to the PE via a stride-permuted access pattern [p, ko, n].

All layout work is done on the HOST: weights/activations are cast and
pre-transposed into [kt, 128, ...] chunks (contraction on partitions),
so the device runs only real GEMM matmuls -- no PE transposes, no casts.

DMA: all three HWDGE queues share ONE engine (~320 GB/s aggregate, fair
round-robin per queue), so the queues are loaded to prioritize bytes by
need-time: phase A's fp8 set (2 MiB) rides the front of two queues while
the sync queue (reserved for yT transposes) stays silent.

Phase structure (per-core, PE-bound throughout):
  - warmup: dummy matmuls on a zeroed tile bridge the runtime preamble
    + DMA fill so the PE's HAM clock-gate opens (1.2 -> 2.4 GHz) before
    real work and never re-throttles.
  - phase A, per 128-row tile: q,k via fp8 DoubleRow, kt4-outer so the
    stationary h8 block is loaded once per 4 matmuls; s = per-head
    rowsum(q*k) on DVE; g = sigmoid(s') on ACT.
  - phase B, per tile: v via bf16 PE; y = g*v on DVE; yT via
    DMA-transpose (jo-halves).
  - phase C, per tile: out = y @ Wo.T via bf16 PE (kt-outer so the
    first half depends only on the first yT half), ACT copy, DMA out.
"""

import numpy as np
import ml_dtypes

import concourse.bass as bass
import concourse.mybir as mybir
import concourse.tile as tile
from concourse import bacc
from concourse.bass_utils import run_bass_kernel_spmd

F32 = mybir.dt.float32
BF16 = mybir.dt.bfloat16
FP8 = mybir.dt.float8e4
BF16_NP = ml_dtypes.bfloat16
FP8_NP = ml_dtypes.float8_e4m3

N_CORES = 8
D = 1024          # model dim
ROWS = 8192       # B*T
M_CORE = ROWS // N_CORES   # rows per core
P = 128           # partitions
KT = D // P       # bf16 contraction chunks (8)
KT4 = KT // 2     # fp8 DoubleRow contraction chunks of 256 (4)
MT = M_CORE // P  # row tiles per core (8)
NCH = 2           # output-column chunks of 512
CHW = D // NCH    # 512
H = 16            # heads
DH = 64           # head dim
HPC = H // NCH    # heads per 512-column chunk (8)
H_SCALE = 16.0    # host pre-scale on h (fp8 copy)
W_SCALE = 1024.0  # host pre-scale on Wq/Wk (fp8)
INV_SQRT_DH = 1.0 / (DH ** 0.5)
SIG_SCALE = INV_SQRT_DH / (H_SCALE * W_SCALE) ** 2
N_WARMUP = 13     # dummy matmuls bridging preamble + DMA fill

_COMPILED = None  # (nc,) cache
LAST_RESULT = None  # BassKernelResults of the most recent run (for test harness)


def _build():
    nc = bacc.Bacc("TRN2", target_bir_lowering=False, debug=False)

    # fp8 stationary for q/k: [kt4, dp, ko, m], d = kt4*256 + ko*128 + dp
    h8_in = nc.dram_tensor("h8", [KT4, P, 2, M_CORE], FP8, kind="ExternalInput")
    # fp8 moving: ko INNERMOST so DoubleRow pairs are byte-adjacent
    wq8_in = nc.dram_tensor("wq8", [KT4, P, D, 2], FP8, kind="ExternalInput")
    wk8_in = nc.dram_tensor("wk8", [KT4, P, D, 2], FP8, kind="ExternalInput")
    # bf16 operands for v/out: [kt, dp, x], d = kt*128 + dp
    h_in = nc.dram_tensor("hT", [KT, P, M_CORE], BF16, kind="ExternalInput")
    wv_in = nc.dram_tensor("wv", [KT, P, D], BF16, kind="ExternalInput")
    wo_in = nc.dram_tensor("wo", [KT, P, D], BF16, kind="ExternalInput")
    # bf16 output halves the end-of-kernel DMA flush; host upcasts
    out = nc.dram_tensor("out", [M_CORE, D], BF16, kind="ExternalOutput")

    with tile.TileContext(nc) as tc:
        with (
            tc.tile_pool(name="wsb", bufs=1) as w_pool,
            tc.tile_pool(name="hsb", bufs=1) as h_pool,
            tc.tile_pool(name="qsb", bufs=2) as qsb_pool,
            tc.tile_pool(name="sp", bufs=2) as sp_pool,
            tc.tile_pool(name="small", bufs=2 * MT) as small_pool,
            tc.tile_pool(name="y", bufs=2) as y_pool,
            tc.tile_pool(name="yT", bufs=MT) as yT_pool,
            tc.tile_pool(name="osb", bufs=2) as o_pool,
            tc.tile_pool(name="ps", bufs=8, space="PSUM") as psum,
        ):
            h8 = h_pool.tile([P, KT4, 2, M_CORE], FP8, tag="h8", name="h8")
            hT = h_pool.tile([P, KT, M_CORE], BF16, tag="hT", name="hT")
            wq8 = w_pool.tile([P, KT4, D, 2], FP8, tag="wq8", name="wq8")
            wk8 = w_pool.tile([P, KT4, D, 2], FP8, tag="wk8", name="wk8")
            wv = w_pool.tile([P, KT, D], BF16, tag="wv", name="wv")
            wo = w_pool.tile([P, KT, D], BF16, tag="wo", name="wo")

            # ---- DMA in, ordered by need-time.  One shared DMA engine
            # round-robins the queues, so: phase A's 3 MiB leads on two
            # queues (chunk-interleaved to match the kt4-outer matmul
            # order), bulk bf16 rides behind, sync stays free for the
            # yT transposes.  Each dma_start also BLOCKS its issuing
            # engine ~the transfer time, so ACT carries only 2 MiB
            # before its gating work starts.
            # 12 critical chunks split 6/6 so round-robin finishes them
            # by ~12us; bulk strictly behind on both queues
            for kt4 in range(KT4 - 1):
                nc.gpsimd.dma_start(out=h8[:, kt4], in_=h8_in[kt4])
                nc.gpsimd.dma_start(out=wk8[:, kt4], in_=wk8_in[kt4])
                nc.scalar.dma_start(out=wq8[:, kt4], in_=wq8_in[kt4])
            nc.scalar.dma_start(out=wq8[:, 3], in_=wq8_in[3])
            nc.scalar.dma_start(out=h8[:, 3], in_=h8_in[3])
            nc.scalar.dma_start(out=wk8[:, 3], in_=wk8_in[3])
            for kt in range(KT):
                nc.gpsimd.dma_start(out=hT[:, kt, :], in_=h_in[kt])
                nc.scalar.dma_start(out=wv[:, kt, :], in_=wv_in[kt])
            for kt in range(KT):
                nc.gpsimd.dma_start(out=wo[:, kt, :], in_=wo_in[kt])

            # ---- HAM warmup: dummy matmuls on a zeroed tile keep the PE
            # busy through its 4096-cycle activity window so the clock
            # un-gates before real work, and bridge the DMA fill.
            z = qsb_pool.tile([P, CHW], BF16, tag="warm", bufs=1)
            nc.vector.memset(z, 0.0)
            zp = psum.tile([P, CHW], F32, tag="ps", name="zp")
            for _ in range(N_WARMUP):
                nc.tensor.matmul(out=zp, lhsT=z[:, :P], rhs=z,
                                 start=True, stop=True)

            # ---- phase A: q,k (fp8 DoubleRow) + gate g per tile ----
            g_tiles = []
            for i in range(MT):
                mi = slice(i * P, (i + 1) * P)
                ps = {}
                for nm in ("q", "k"):
                    for jo in range(NCH):
                        ps[(nm, jo)] = psum.tile([P, CHW], F32, tag="ps",
                                                 name="pt")
                # kt4-outer: one 256-col LDWEIGHTS feeds 4 matmuls
                for kt4 in range(KT4):
                    for w8, nm in ((wq8, "q"), (wk8, "k")):
                        for jo in range(NCH):
                            nc.tensor.matmul(
                                out=ps[(nm, jo)],
                                lhsT=h8[:, kt4, :, mi],
                                rhs=w8[:, kt4, jo * CHW:(jo + 1) * CHW, :]
                                    .rearrange("p n k -> p k n"),
                                start=(kt4 == 0),
                                stop=(kt4 == KT4 - 1),
                                perf_mode=mybir.MatmulPerfMode.DoubleRow,
                                skip_group_check=True,
                            )

                # s[m, h] = sum_{d in head} q*k ; g = sigmoid(s * scale)
                # (DVE can read only one PSUM operand: stage q in SBUF)
                s = small_pool.tile([P, H], F32, tag="s")
                for jo in range(NCH):
                    qsb = qsb_pool.tile([P, CHW], BF16, tag="qsb")
                    nc.scalar.copy(out=qsb, in_=ps[("q", jo)])
                    sp = sp_pool.tile([P, CHW], F32, tag="sp")
                    nc.vector.tensor_mul(out=sp, in0=qsb, in1=ps[("k", jo)])
                    nc.vector.reduce_sum(
                        out=s[:, jo * HPC:(jo + 1) * HPC],
                        in_=sp.rearrange("p (h d) -> p h d", d=DH),
                        axis=mybir.AxisListType.X,
                    )
                g = small_pool.tile([P, H], F32, tag="g")
                nc.scalar.activation(
                    out=g, in_=s,
                    func=mybir.ActivationFunctionType.Sigmoid,
                    scale=SIG_SCALE,
                )
                g_tiles.append(g)

            # ---- phase B: v (bf16) + y = g*v + yT per tile ----
            yT_tiles = []
            for i in range(MT):
                mi = slice(i * P, (i + 1) * P)
                vps = [psum.tile([P, CHW], F32, tag="ps", name="vp")
                       for _ in range(NCH)]
                for jo in range(NCH):
                    for kt in range(KT):
                        nc.tensor.matmul(
                            out=vps[jo],
                            lhsT=hT[:, kt, mi],
                            rhs=wv[:, kt, jo * CHW:(jo + 1) * CHW],
                            start=(kt == 0),
                            stop=(kt == KT - 1),
                        )
                y = y_pool.tile([P, D], BF16, tag="y")
                g = g_tiles[i]
                for jo in range(NCH):
                    g_sl = g[:, jo * HPC:(jo + 1) * HPC]
                    g_bc = bass.AP(
                        tensor=g_sl.tensor, offset=g_sl.offset,
                        ap=[*g_sl.ap, [0, DH]],
                    )
                    nc.vector.tensor_mul(
                        out=y[:, jo * CHW:(jo + 1) * CHW].rearrange(
                            "p (h d) -> p h d", d=DH),
                        in0=vps[jo].rearrange("p (h d) -> p h d", d=DH),
                        in1=g_bc,
                    )
                # transpose per jo-half so phase C's first kt-group can
                # start before the second half of y is even computed
                yT = yT_pool.tile([P, KT, P], BF16, tag="yT")
                for jo in range(NCH):
                    kh = slice(jo * (KT // NCH), (jo + 1) * (KT // NCH))
                    nc.sync.dma_start_transpose(
                        out=yT[:, kh, :],
                        in_=y[:, jo * CHW:(jo + 1) * CHW])
                yT_tiles.append(yT)

            # ---- phase C: out = y @ Wo.T per tile ----
            for i in range(MT):
                mi = slice(i * P, (i + 1) * P)
                osb = o_pool.tile([P, D], BF16, tag="osb")
                ops = [psum.tile([P, CHW], F32, tag="ps", name="op")
                       for _ in range(NCH)]
                # kt-outer: the kt<4 matmuls depend only on the first
                # yT half-transpose, trimming the end-of-kernel tail
                for kt in range(KT):
                    for jo in range(NCH):
                        nc.tensor.matmul(
                            out=ops[jo],
                            lhsT=yT_tiles[i][:, kt, :],
                            rhs=wo[:, kt, jo * CHW:(jo + 1) * CHW],
                            start=(kt == 0),
                            stop=(kt == KT - 1),
                            skip_group_check=True,
                        )
                for jo in range(NCH):
                    nc.scalar.copy(out=osb[:, jo * CHW:(jo + 1) * CHW],
                                   in_=ops[jo])
                    nc.gpsimd.dma_start(
                        out=out[mi, jo * CHW:(jo + 1) * CHW],
                        in_=osb[:, jo * CHW:(jo + 1) * CHW])

    nc.compile()
    return nc


def kernel(hidden_states, m_gate, alpha_scale, Wq, Wk, Wv, Wo, Wa, ba, mix_logit,
           **_unused):
    global _COMPILED, LAST_RESULT
    if _COMPILED is None:
        _COMPILED = _build()
    nc = _COMPILED

    h = np.asarray(hidden_states, dtype=np.float32).reshape(ROWS, D)
    h_bf = h.astype(BF16_NP)

    def fp8_chunks(x, scale):
        # x [n, d] -> [kt4, ko, dp, n] fp8, d = kt4*256 + ko*128 + dp
        xt = np.clip(np.asarray(x, np.float32).T * scale, -240.0, 240.0)
        return xt.reshape(KT4, 2, P, xt.shape[1]).astype(FP8_NP)

    def to_fp8_stat(x, scale):
        # stationary: [kt4, dp, ko, n] (ko-major pairs)
        return np.ascontiguousarray(fp8_chunks(x, scale).transpose(0, 2, 1, 3))

    def to_fp8_mov(x, scale):
        # moving: [kt4, dp, n, ko] (ko-INNERMOST adjacent pairs)
        return np.ascontiguousarray(fp8_chunks(x, scale).transpose(0, 2, 3, 1))

    def to_bf16_T(w):
        # W.T chunked [kt, dp, j]; d = kt*128 + dp on partitions
        wt = np.ascontiguousarray(np.asarray(w, np.float32).T.astype(BF16_NP))
        return wt.reshape(KT, P, D)

    wmats = {
        "wq8": to_fp8_mov(Wq, W_SCALE),
        "wk8": to_fp8_mov(Wk, W_SCALE),
        "wv": to_bf16_T(Wv),
        "wo": to_bf16_T(Wo),
    }

    in_maps = []
    for c in range(N_CORES):
        hc = h[c * M_CORE:(c + 1) * M_CORE]
        hcb = np.ascontiguousarray(h_bf[c * M_CORE:(c + 1) * M_CORE].T)
        in_maps.append({
            "h8": to_fp8_stat(hc, H_SCALE),
            "hT": hcb.reshape(KT, P, M_CORE),
            **wmats,
        })

    res = run_bass_kernel_spmd(nc, in_maps, core_ids=list(range(N_CORES)))
    LAST_RESULT = res
    out = np.concatenate(
        [res.results[c]["out"].astype(np.float32) for c in range(N_CORES)],
        axis=0)
    B, T = 4, 2048
    return out.reshape(B, T, D)
